# revision 37
# baseline (speedup 1.0000x reference)
"""GAT (2-layer, 4-head then 1-head) Bass kernel for TRN2, 8-way graph-parallel.

v3 design (per core, cores own contiguous dst-node shards of Nc nodes):
  - build1: table1[n] = [h1 bf16 x128 | fp32 a_s(4) a_d(4) | pad] for ALL n
    (replicated dense matmul pass); own-shard h/scores kept in SBUF (own1_sb).
  - aggregation: edges (NO self-loops) sorted by dst into 128-dst windows,
    grouped 2 windows per group; per group FOUR dma_gathers (lo/hi table
    halves, each split in two) on 4 SWDGE queues into 4 separate row tiles
    (concurrent ring drain); one-hot matrices oh [e,sl,d] and ohT [d,sl,e]
    are HOST-PRECOMPUTED in fp8 and DMA'd from DRAM (no is_eq, no PE
    transposes, no PSUM round trips); per-edge a_d via per-slot PE matmuls
    (ohT fp8 lhsT @ adw); exp(lrelu(.)) = max(exp(x), exp(0.2x)) on ACT;
    messages scattered to dst windows via PE matmul accumulation in PSUM
    (oh fp8 lhsT x bf16 msg). Self-loops handled densely at window drain.
  - NO build2: at L1 window drain, h2 = relu(o1) @ W2 plus a_s2/a_d2 scores
    are computed per window (transpose + one [128,66] matmul) and written
    as table2 rows [h2 bf16 x64 | fp32 a_s2 a_d2 | pad] to the local shard;
    AllGather of [Nc,128] shards produces the full table2; own2_sb filled
    at drain. Layer 2 aggregation = same machinery, heads=1, 64 features.
Output: per-core dst shard [Nc, 64] fp32; host concatenates, adds b2.
"""

import math
from contextlib import ExitStack

import numpy as np
import ml_dtypes

import concourse.bass as bass
import concourse.mybir as mybir
import concourse.tile as tile

P = 128
FP32 = mybir.dt.float32
BF16 = mybir.dt.bfloat16
FP8 = mybir.dt.float8e4
I16 = mybir.dt.int16
AF = mybir.ActivationFunctionType
OP = mybir.AluOpType

NEG_SLOPE = 0.2
N_FULL = 50000
N_CORES = 8

LAST_RESULT = None

FP8_ONE = np.array([1.0], ml_dtypes.float8_e4m3).view(np.uint8)[0]


# ----------------------------------------------------------------------------
# Host-side planning (pure index/structure work; no tensor-value compute)
# ----------------------------------------------------------------------------

class Plan:
    pass


def table1_pos(n, N):
    """node id -> position in table1 (interleaved for batched 1024-row writes)."""
    n = np.asarray(n, np.int64)
    TB = 1024
    ntile1 = N // TB
    t = n // TB
    off = n % TB
    nrem_blocks = (N - ntile1 * TB) // P
    pos1 = np.where(
        t < ntile1,
        t * TB + (off % P) * 8 + off // P,
        np.where(n < ntile1 * TB + nrem_blocks * P,
                 ntile1 * TB + (off % P) * nrem_blocks + off // P,
                 n))
    return pos1


def make_plan(edge_index: np.ndarray, N: int, n_cores: int, group_windows: int = 2):
    p = Plan()
    assert N % n_cores == 0
    Nc = N // n_cores
    n_win = math.ceil(Nc / P)
    split = (N // 2 + P - 1) // P * P
    assert split < 32768 and (N - split) < 32768

    src = edge_index[0].astype(np.int64)
    dst = edge_index[1].astype(np.int64)

    pos1 = table1_pos(src, N)
    pos2 = src  # table2 rows are written in natural node order
    assert int(pos1.max()) < N

    core = dst // Nc
    win = (dst % Nc) // P
    wloc = (dst % Nc) % P
    # half assignment valid for BOTH tables: pos1 interleaving moves a row by
    # < 1024, so node-id criterion with hysteresis; pos2 = node id directly.
    is_hi = (src >= split + 1024).astype(np.int64)
    assert int(pos1[is_hi == 0].max()) < 32768
    assert int(pos2[is_hi == 0].max()) < 32768
    assert int(pos1[is_hi == 1].min()) >= split
    assert int(pos2[is_hi == 1].min()) >= split

    order = np.lexsort((src, is_hi, win, core))
    so_p1, so_p2, so_core, so_win, so_wloc, so_hi = (
        pos1[order], pos2[order], core[order], win[order], wloc[order],
        is_hi[order])

    counts = np.zeros((n_cores, n_win, 2), dtype=np.int64)
    np.add.at(counts, (so_core, so_win, so_hi), 1)
    cpw = np.ceil(counts / P).astype(np.int64).max(axis=0)  # [n_win, 2]

    groups = []
    slot_global = 0
    lo_col = 0
    hi_col = 0
    for g0 in range(0, n_win, group_windows):
        ws = list(range(g0, min(g0 + group_windows, n_win)))
        g = Plan()
        g.windows = ws
        g.slot0 = slot_global
        g.lo_n = int(sum(cpw[w, 0] for w in ws))
        g.hi_n = int(sum(cpw[w, 1] for w in ws))
        g.n_slots = g.lo_n + g.hi_n
        g.lo_col0 = lo_col
        g.hi_col0 = hi_col
        g.win_runs = {}
        loff, hoff = 0, g.lo_n
        for w in ws:
            g.win_runs[w] = (loff, int(cpw[w, 0]), hoff, int(cpw[w, 1]))
            loff += int(cpw[w, 0])
            hoff += int(cpw[w, 1])
        lo_col += g.lo_n * (P // 16)
        hi_col += g.hi_n * (P // 16)
        slot_global += g.n_slots
        # sub-gather slot ranges (group-local): lo and hi each split in 2
        # (exactly one gather per SWDGE queue per group — the per-queue DMA
        # completion semaphore cannot distinguish two in-flight gathers)
        subs = []
        for base, n, half in ((0, g.lo_n, 0), (g.lo_n, g.hi_n, 1)):
            nch = min(2, n) if n else 0
            for j in range(nch):
                s0 = base + (n * j) // nch
                s1 = base + (n * (j + 1)) // nch
                if s1 > s0:
                    subs.append((s0, s1, half))
        g.subs = subs
        groups.append(g)

    S = slot_global
    TOT_LO = lo_col * 16
    TOT_HI = hi_col * 16

    idx_lo = np.zeros((2, n_cores, 16, TOT_LO // 16), dtype=np.int16)
    idx_hi = np.zeros((2, n_cores, 16, TOT_HI // 16), dtype=np.int16)
    oh_u8 = np.zeros((n_cores, P, S, P), dtype=np.uint8)    # [c, e, sl, d]
    ohT_u8 = np.zeros((n_cores, P, S, P), dtype=np.uint8)   # [c, d, sl, e]

    start = {}
    pos = 0
    for c in range(n_cores):
        for w in range(n_win):
            for h in range(2):
                cnt = int(counts[c, w, h])
                start[(c, w, h)] = (pos, cnt)
                pos += cnt
    assert pos == len(so_p1)

    for c in range(n_cores):
        for g in groups:
            for w in g.windows:
                lo0, lon, hi0, hin = g.win_runs[w]
                for h in (0, 1):
                    base_pos, cnt = start[(c, w, h)]
                    run0 = lo0 if h == 0 else hi0
                    runn = lon if h == 0 else hin
                    for j in range(runn):
                        s_loc = run0 + j
                        s = g.slot0 + s_loc
                        lo_e = j * P
                        n_e = min(P, cnt - lo_e) if cnt > lo_e else 0
                        sl_ = slice(base_pos + lo_e, base_pos + lo_e + n_e)
                        wl = so_wloc[sl_]
                        if n_e > 0:
                            lanes = np.arange(n_e)
                            oh_u8[c, lanes, s, wl] = FP8_ONE
                            ohT_u8[c, wl, s, lanes] = FP8_ONE
                        for li, so_pos in ((0, so_p1), (1, so_p2)):
                            gidx = np.zeros((P,), np.int64)
                            gidx[:n_e] = so_pos[sl_] - (split if h == 1 else 0)
                            if h == 0:
                                col0 = g.lo_col0 + s_loc * (P // 16)
                                tgt = idx_lo
                            else:
                                col0 = g.hi_col0 + (s_loc - g.lo_n) * (P // 16)
                                tgt = idx_hi
                            tgt[li, c, :, col0:col0 + P // 16] = \
                                gidx.astype(np.int16).reshape(P // 16, 16).T

    # per-core real index counts per (group, sub): pads are strictly trailing
    # within each sub-gather stream (requires group_windows == 1: one
    # (win, half) run per half per group).
    if group_windows == 1:
        subcnt = np.zeros((n_cores, len(groups) * 4), np.int32)
        for c in range(n_cores):
            for gi, g in enumerate(groups):
                w = g.windows[0]
                for k, (s0, s1, half) in enumerate(g.subs):
                    run0 = 0 if half == 0 else g.lo_n
                    cnt = int(counts[c, w, half])
                    real = min(max(cnt - (s0 - run0) * P, 0),
                               (s1 - s0) * P)
                    r16 = (real + 15) // 16 * 16
                    subcnt[c, gi * 4 + k] = min(max(r16, 16), (s1 - s0) * P)
        p.subcnt = subcnt
    else:
        p.subcnt = None

    p.N, p.n_cores, p.Nc, p.n_win, p.split = N, n_cores, Nc, n_win, split
    p.groups, p.S, p.TOT_LO, p.TOT_HI = groups, S, TOT_LO, TOT_HI
    p.idx_lo = np.tile(idx_lo, (1, 1, 8, 1))
    p.idx_hi = np.tile(idx_hi, (1, 1, 8, 1))
    p.oh = oh_u8.reshape(n_cores, P, S * P).view(ml_dtypes.float8_e4m3)
    p.ohT = ohT_u8.reshape(n_cores, P, S * P).view(ml_dtypes.float8_e4m3)
    p.win_ndst = [min(P, Nc - w * P) for w in range(n_win)]
    p.submax = max(s1 - s0 for g in groups for (s0, s1, _h) in g.subs)
    return p


# ----------------------------------------------------------------------------
# Device program emitter
# ----------------------------------------------------------------------------

def emit_gat(tc, outs, ins, plan):
    nc = tc.nc
    N, Nc, n_win, split = plan.N, plan.Nc, plan.n_win, plan.split
    n_cores = plan.n_cores
    S = plan.S
    HC, OUT, H1 = 128, 64, 4
    Smax = max(g.n_slots for g in plan.groups)
    SUBMAX = plan.submax

    xT = ins["xT"]            # [128, N] bf16
    xT_own = ins["xT_own"]    # [128, Nc] bf16
    W1aug = ins["W1aug"]      # [128, 192] bf16
    W2m = ins["W2m"]          # [128, 66] bf16  [W2 | m2s | m2d]
    ident_in = ins["ident"]   # [128, 128] bf16 identity
    oh_in = ins["oh"]         # [128, S*128] fp8   oh[e, sl, d]
    ohT_in = ins["ohT"]       # [128, S*128] fp8   ohT[d, sl, e]
    out2 = outs["out2"]       # [Nc, 64] fp32

    ctx = ExitStack()
    with ctx:
        dram = ctx.enter_context(tc.tile_pool(name="dram", bufs=1, space="DRAM"))
        cpool = ctx.enter_context(tc.tile_pool(name="consts", bufs=1))

        table1 = dram.tile([N, 256], BF16, name="table1")
        t2own = dram.tile([Nc, 128], BF16, name="t2own")
        t2full = dram.tile([N, 128], BF16, name="t2full", addr_space="Shared")

        # ---- constants to SBUF
        w1_sb = cpool.tile([P, 192], BF16, name="w1_sb")
        nc.sync.dma_start(out=w1_sb[:], in_=W1aug[:])
        w2m_sb = cpool.tile([P, 66], BF16, name="w2m_sb")
        nc.sync.dma_start(out=w2m_sb[:], in_=W2m[:])
        ident_sb = cpool.tile([P, P], BF16, name="ident_sb")
        nc.sync.dma_start(out=ident_sb[:], in_=ident_in[:])
        idxlo_sb = {}
        idxhi_sb = {}
        for li in (1, 2):
            idxlo_sb[li] = cpool.tile([P, plan.TOT_LO // 16], I16,
                                      name=f"idxlo{li}_sb")
            nc.scalar.dma_start(out=idxlo_sb[li][:], in_=ins[f"idx_lo{li}"][:])
            idxhi_sb[li] = cpool.tile([P, plan.TOT_HI // 16], I16,
                                      name=f"idxhi{li}_sb")
            nc.scalar.dma_start(out=idxhi_sb[li][:], in_=ins[f"idx_hi{li}"][:])
        subcnt_sb = None
        if plan.subcnt is not None:
            ng4 = plan.subcnt.shape[1]
            subcnt_sb = cpool.tile([1, ng4], mybir.dt.int32, name="subcnt_sb")
            nc.sync.dma_start(out=subcnt_sb[:], in_=ins["subcnt"][:])
        own1_sb = cpool.tile([P, n_win, 256], BF16, name="own1_sb")
        own2_sb = cpool.tile([P, n_win, 128], BF16, name="own2_sb")
        own1f = own1_sb[:].bitcast(FP32)   # [P, n_win, 128]
        own2f = own2_sb[:].bitcast(FP32)   # [P, n_win, 64]

        def build_table1():
            """xT @ W1aug -> table1 rows (all N) + own1_sb (own shard)."""
            RC, F, NS, SC0 = 192, HC, 8, 64
            with tc.tile_pool(name="bld1", bufs=3) as bpool, \
                 tc.tile_pool(name="bps1", bufs=2, space="PSUM") as bps:

                # own pass -> own1_sb
                nc.vector.memset(own1_sb[:, n_win - 1, :], 0.0)
                nblk = math.ceil(Nc / P)
                tog = True
                for t0 in range(0, nblk, 8):
                    nbl = min(8, nblk - t0)
                    nb_last = min(P, Nc - (t0 + nbl - 1) * P)
                    ncols = (nbl - 1) * P + nb_last
                    c0 = t0 * P
                    xo = bpool.tile([P, 1024], BF16, name="xo", tag="xt")
                    eng = nc.sync if tog else nc.scalar
                    eng.dma_start(out=xo[:, :ncols], in_=xT_own[:, c0:c0 + ncols])
                    for h0 in range(0, nbl, 4):
                        hn = min(4, nbl - h0)
                        pso = bps.tile([P, 4, 512], FP32, name="pso", tag="ps")
                        for j in range(h0, h0 + hn):
                            nb = P if j < nbl - 1 else nb_last
                            nc.tensor.matmul(out=pso[:nb, j - h0, 0:RC],
                                             lhsT=xo[:, j * P:j * P + nb],
                                             rhs=w1_sb[:], start=True, stop=True)
                        for j in range(h0, h0 + hn):
                            w = t0 + j
                            nb = P if j < nbl - 1 else nb_last
                            nc.scalar.activation(out=own1_sb[:nb, w, 0:F],
                                                 in_=pso[:nb, j - h0, 0:F],
                                                 func=AF.Copy)
                            nc.vector.tensor_copy(
                                out=own1f[:nb, w, SC0:SC0 + NS],
                                in_=pso[:nb, j - h0, F:F + NS])
                    tog = not tog

                # main pass over all N rows
                nblk = math.ceil(N / P)
                tog = False
                for t0 in range(0, nblk, 8):
                    nbl = min(8, nblk - t0)
                    nb_last = min(P, N - (t0 + nbl - 1) * P)
                    ncols = (nbl - 1) * P + nb_last
                    row0 = t0 * P
                    xt = bpool.tile([P, 1024], BF16, name="xt", tag="xt")
                    eng = nc.sync if tog else nc.scalar
                    eng.dma_start(out=xt[:, :ncols],
                                  in_=xT[:, row0:row0 + ncols])
                    t1 = bpool.tile([P, 8, 144], BF16, name="t1", tag="t1")
                    for h0 in range(0, nbl, 4):
                        hn = min(4, nbl - h0)
                        ps = bps.tile([P, 4, 512], FP32, name="ps", tag="ps")
                        for j in range(h0, h0 + hn):
                            nb = P if j < nbl - 1 else nb_last
                            nc.tensor.matmul(out=ps[:nb, j - h0, 0:RC],
                                             lhsT=xt[:, j * P:j * P + nb],
                                             rhs=w1_sb[:], start=True, stop=True)
                        nc.scalar.activation(out=t1[:, h0:h0 + hn, 0:F],
                                             in_=ps[:, :hn, 0:F], func=AF.Copy)
                        t1f = t1[:].bitcast(FP32)
                        nc.vector.tensor_copy(
                            out=t1f[:, h0:h0 + hn, SC0:SC0 + NS],
                            in_=ps[:, :hn, F:F + NS])
                    # write only the 288B of content per 512B row
                    nfull = nbl if nb_last == P else nbl - 1
                    if nfull > 0:
                        dst = table1[row0:row0 + nfull * P, 0:144] \
                            .rearrange("(p j) f -> p j f", j=nfull)
                        eng2 = nc.scalar if tog else nc.sync
                        eng2.dma_start(out=dst, in_=t1[:, :nfull, :])
                    if nb_last < P:
                        r0 = row0 + nfull * P
                        eng.dma_start(out=table1[r0:r0 + nb_last, 0:144],
                                      in_=t1[:nb_last, nfull, :])
                    tog = not tog

        def emit_ag(r0, r1):
            nc.gpsimd.collective_compute(
                "AllGather", OP.bypass,
                replica_groups=[list(range(n_cores))],
                ins=[t2own[r0:r1, :]],
                outs=[t2full[:].rearrange("(c r) f -> c r f", c=n_cores)
                      [:, r0:r1, :]],
            )

        def emit_layer(layer, ag_chunks=None):
            ag_chunks = ag_chunks or {}
            H = H1 if layer == 1 else 1
            F = HC if layer == 1 else OUT
            ROW = 256 if layer == 1 else 128
            ASF = 64 if layer == 1 else 32   # fp32 col of a_s in table rows
            tab = table1 if layer == 1 else t2full
            own_sb = own1_sb if layer == 1 else own2_sb
            ownf = own1f if layer == 1 else own2f
            OSC = 64 if layer == 1 else 32   # fp32 col of a_s in own rows
            idxl = idxlo_sb[layer]
            idxh = idxhi_sb[layer]

            with tc.tile_pool(name=f"rows{layer}", bufs=4) as rpool, \
                 tc.tile_pool(name=f"oh{layer}",
                              bufs=(3 if layer == 1 else 6)) as ohpool, \
                 tc.tile_pool(name=f"sp{layer}", bufs=2) as spool, \
                 tc.tile_pool(name=f"dp{layer}", bufs=3) as dpool, \
                 tc.tile_pool(name=f"ado{layer}", bufs=2, space="PSUM") as adops, \
                 tc.tile_pool(name=f"wps{layer}", bufs=2, space="PSUM") as wps, \
                 tc.tile_pool(name=f"tps{layer}", bufs=2, space="PSUM") as tps:

                cntregs = None

                for gi, g in enumerate(plan.groups):
                    Sg = g.n_slots
                    # one-hot tiles from DRAM (fp8), host-precomputed
                    oh_sb = ohpool.tile([P, Sg, P], FP8, name="oh", tag="oh",
                                        padded_shape=[P, Smax, P])
                    nc.sync.dma_start(
                        out=oh_sb[:],
                        in_=oh_in[:, g.slot0 * P:(g.slot0 + Sg) * P])
                    ohT_sb = ohpool.tile([P, Sg, P], FP8, name="ohT", tag="ohT",
                                         padded_shape=[P, Smax, P])
                    nc.scalar.dma_start(
                        out=ohT_sb[:],
                        in_=ohT_in[:, g.slot0 * P:(g.slot0 + Sg) * P])

                    # 4 sub-gathers on 4 SWDGE queues into 4 tiles
                    rtiles = []
                    for k, (s0, s1, half) in enumerate(g.subs):
                        ns = s1 - s0
                        rt = rpool.tile([P, ns, ROW], BF16, name=f"r{k}",
                                        tag=f"r{k}",
                                        padded_shape=[P, SUBMAX, ROW])
                        cnt_reg = ns * P
                        if half == 0:
                            src_ap = tab[0:split, :]
                            idx_ap = idxl[:, g.lo_col0 + s0 * 8:
                                          g.lo_col0 + s1 * 8]
                        else:
                            src_ap = tab[split:N, :]
                            h0 = s0 - g.lo_n
                            h1_ = s1 - g.lo_n
                            idx_ap = idxh[:, g.hi_col0 + h0 * 8:
                                          g.hi_col0 + h1_ * 8]
                        nc.gpsimd.dma_gather(
                            out_ap=rt[:],
                            in_ap=src_ap,
                            idxs_ap=idx_ap,
                            num_idxs=ns * P,
                            num_idxs_reg=cnt_reg,
                            elem_size=ROW,
                            single_packet=True,
                            queue_num=(k + gi) % 4,
                        )
                        rtiles.append((s0, s1, rt))

                    def rtile(sl):
                        for (s0, s1, rt) in rtiles:
                            if s0 <= sl < s1:
                                return rt, sl - s0
                        raise AssertionError(sl)

                    # per-window a_d weights
                    adw = {}
                    for w in g.windows:
                        aw = dpool.tile([P, H], BF16, name="adw", tag="adw",
                                        padded_shape=[P, H1])
                        nc.scalar.activation(
                            out=aw[:], in_=ownf[:, w, OSC + H:OSC + 2 * H],
                            func=AF.Copy)
                        adw[w] = aw
                    # per-edge a_d via per-slot PE matmuls (fp8 lhsT)
                    ado = adops.tile([P, Sg * H], FP32, name="ado", tag="ado",
                                     padded_shape=[P, Smax * H1])
                    for w in g.windows:
                        lo0, lon, hi0, hin = g.win_runs[w]
                        for s0_, sn in ((lo0, lon), (hi0, hin)):
                            for sl in range(s0_, s0_ + sn):
                                nc.tensor.matmul(
                                    out=ado[:, sl * H:(sl + 1) * H],
                                    lhsT=ohT_sb[:, sl, :], rhs=adw[w][:],
                                    start=True, stop=True)

                    # per sub-tile: scores -> expt -> msg
                    mtiles = []
                    for k, (s0, s1, rt) in enumerate(rtiles):
                        ns = s1 - s0
                        rows_f = rt[:].bitcast(FP32)   # [P, ns, ROW//2]
                        e_t = spool.tile([P, ns * H], FP32, name=f"e{k}",
                                         tag=f"e{k}",
                                         padded_shape=[P, SUBMAX * H1])
                        nc.vector.tensor_tensor(
                            out=e_t[:].rearrange("p (s h) -> p s h", h=H),
                            in0=rows_f[:, :, ASF:ASF + H],
                            in1=ado[:, s0 * H:s1 * H]
                                .rearrange("p (s h) -> p s h", h=H),
                            op=OP.add)
                        eA = spool.tile([P, ns * H], FP32, name=f"eA{k}",
                                        tag=f"eA{k}",
                                        padded_shape=[P, SUBMAX * H1])
                        nc.scalar.activation(out=eA[:], in_=e_t[:], func=AF.Exp)
                        eB = spool.tile([P, ns * H], FP32, name=f"eB{k}",
                                        tag=f"eB{k}",
                                        padded_shape=[P, SUBMAX * H1])
                        nc.scalar.activation(out=eB[:], in_=e_t[:], func=AF.Exp,
                                             scale=NEG_SLOPE)
                        expt = spool.tile([P, ns, H], BF16, name=f"ex{k}",
                                          tag=f"ex{k}",
                                          padded_shape=[P, SUBMAX, H1])
                        nc.vector.tensor_tensor(
                            out=expt[:].rearrange("p s h -> p (s h)"),
                            in0=eA[:], in1=eB[:], op=OP.max)
                        msg = ohpool.tile([P, ns, F + H], BF16, name=f"m{k}",
                                          tag=f"m{k}",
                                          padded_shape=[P, SUBMAX, HC + H1])
                        nc.vector.tensor_tensor(
                            out=msg[:, :, 0:F]
                                .rearrange("p s (h c) -> p s h c", h=H),
                            in0=rt[:, :, 0:F]
                                .rearrange("p s (h c) -> p s h c", h=H),
                            in1=expt[:, :, :, None]
                                .to_broadcast([P, ns, H, F // H]),
                            op=OP.mult)
                        nc.scalar.activation(out=msg[:, :, F:F + H], in_=expt[:],
                                             func=AF.Copy)
                        mtiles.append((s0, s1, msg))

                    def mtile(sl):
                        for (s0, s1, mt) in mtiles:
                            if s0 <= sl < s1:
                                return mt, sl - s0
                        raise AssertionError(sl)

                    for w in g.windows:
                        Dw = plan.win_ndst[w]
                        lo0, lon, hi0, hin = g.win_runs[w]
                        slots = list(range(lo0, lo0 + lon)) + \
                                list(range(hi0, hi0 + hin))
                        psw = wps.tile([P, F + H], FP32, name="psw", tag="psw",
                                       padded_shape=[P, HC + H1])
                        for si, sl in enumerate(slots):
                            mt, ml = mtile(sl)
                            nc.tensor.matmul(out=psw[:], lhsT=oh_sb[:, sl, :],
                                             rhs=mt[:, ml, :],
                                             start=(si == 0),
                                             stop=(si == len(slots) - 1))
                        # self-loop from own rows
                        eo_p = dpool.tile([P, H], FP32, name="eo_p", tag="eo_p",
                                          padded_shape=[P, H1])
                        nc.vector.tensor_tensor(
                            out=eo_p[:], in0=ownf[:, w, OSC:OSC + H],
                            in1=ownf[:, w, OSC + H:OSC + 2 * H], op=OP.add)
                        eoA = dpool.tile([P, H], FP32, name="eoA", tag="eoA",
                                         padded_shape=[P, H1])
                        nc.scalar.activation(out=eoA[:], in_=eo_p[:], func=AF.Exp)
                        eoB = dpool.tile([P, H], FP32, name="eoB", tag="eoB",
                                         padded_shape=[P, H1])
                        nc.scalar.activation(out=eoB[:], in_=eo_p[:], func=AF.Exp,
                                             scale=NEG_SLOPE)
                        eo = dpool.tile([P, H], FP32, name="eo", tag="eo",
                                        padded_shape=[P, H1])
                        nc.vector.tensor_tensor(out=eo[:], in0=eoA[:], in1=eoB[:],
                                                op=OP.max)
                        mo = dpool.tile([P, F], BF16, name="mo", tag="mo",
                                        padded_shape=[P, HC])
                        nc.vector.tensor_tensor(
                            out=mo[:].rearrange("p (h c) -> p h c", h=H),
                            in0=own_sb[:, w, 0:F]
                                .rearrange("p (h c) -> p h c", h=H),
                            in1=eo[:, :, None].to_broadcast([P, H, F // H]),
                            op=OP.mult)
                        # drain
                        den = dpool.tile([P, H], FP32, name="den", tag="den",
                                         padded_shape=[P, H1])
                        nc.vector.tensor_tensor(out=den[:], in0=psw[:, F:F + H],
                                                in1=eo[:], op=OP.add)
                        rec = dpool.tile([P, H], FP32, name="rec", tag="rec",
                                         padded_shape=[P, H1])
                        nc.vector.reciprocal(out=rec[:], in_=den[:])
                        o1 = dpool.tile([P, F], FP32, name="o1", tag="o1",
                                        padded_shape=[P, HC])
                        nc.vector.tensor_tensor(out=o1[:], in0=psw[:, 0:F],
                                                in1=mo[:], op=OP.add)
                        o1m = dpool.tile([P, F], FP32, name="o1m", tag="o1m",
                                         padded_shape=[P, HC])
                        nc.vector.tensor_tensor(
                            out=o1m[:].rearrange("p (h c) -> p h c", h=H),
                            in0=o1[:].rearrange("p (h c) -> p h c", h=H),
                            in1=rec[:, :, None].to_broadcast([P, H, F // H]),
                            op=OP.mult)
                        if layer == 1:
                            # relu -> h2/table2 row for this window
                            o1b = dpool.tile([P, F], BF16, name="o1b", tag="o1b")
                            nc.scalar.activation(out=o1b[:], in_=o1m[:],
                                                 func=AF.Relu)
                            pst = tps.tile([P, P], BF16, name="pst", tag="pst")
                            nc.tensor.transpose(out=pst[:], in_=o1b[:],
                                                identity=ident_sb[:])
                            o1t = dpool.tile([P, P], BF16, name="o1t", tag="o1t")
                            nc.scalar.activation(out=o1t[:], in_=pst[:],
                                                 func=AF.Copy)
                            h2ps = tps.tile([P, 66], FP32, name="h2ps",
                                            tag="h2ps")
                            nc.tensor.matmul(out=h2ps[:], lhsT=o1t[:],
                                             rhs=w2m_sb[:], start=True,
                                             stop=True)
                            nc.scalar.activation(out=own2_sb[:, w, 0:OUT],
                                                 in_=h2ps[:, 0:OUT],
                                                 func=AF.Copy)
                            nc.vector.tensor_copy(out=own2f[:, w, 32:34],
                                                  in_=h2ps[:, OUT:OUT + 2])
                            nc.sync.dma_start(
                                out=t2own[w * P:w * P + Dw, :],
                                in_=own2_sb[:Dw, w, :])
                        else:
                            nc.sync.dma_start(out=out2[w * P:w * P + Dw, :],
                                              in_=o1m[:Dw, :])

                    if gi in ag_chunks:
                        emit_ag(*ag_chunks[gi])

        # ---------------- phases
        build_table1()
        emit_layer(1)
        emit_ag(0, Nc)
        emit_layer(2)


# ----------------------------------------------------------------------------
# Host input construction
# ----------------------------------------------------------------------------

def build_host_inputs(plan, x, W1, att_src1, att_dst1, W2, att_src2, att_dst2):
    bf = ml_dtypes.bfloat16
    HID = 32
    H1 = att_src1.shape[0]
    m1s = np.stack([W1[:, h * HID:(h + 1) * HID] @ att_src1[h]
                    for h in range(H1)], axis=1)
    m1d = np.stack([W1[:, h * HID:(h + 1) * HID] @ att_dst1[h]
                    for h in range(H1)], axis=1)
    m2s = W2 @ att_src2[0]
    m2d = W2 @ att_dst2[0]
    W1p = np.zeros((128, 192), np.float32)
    W1p[:, 0:128] = W1
    W1p[:, 128:132] = m1s
    W1p[:, 132:136] = m1d
    W1aug = W1p.astype(bf)
    W2p = np.zeros((128, 66), np.float32)
    W2p[:, 0:64] = W2
    W2p[:, 64] = m2s
    W2p[:, 65] = m2d
    W2m = W2p.astype(bf)

    xT = np.ascontiguousarray(x.T).astype(bf)  # [128, N]
    ident = np.eye(128, dtype=np.float32).astype(bf)

    shared = dict(xT=xT, W1aug=W1aug, W2m=W2m, ident=ident)
    in_maps = []
    for c in range(plan.n_cores):
        m = dict(shared)
        m["xT_own"] = np.ascontiguousarray(xT[:, c * plan.Nc:(c + 1) * plan.Nc])
        m["idx_lo1"] = plan.idx_lo[0, c]
        m["idx_hi1"] = plan.idx_hi[0, c]
        m["idx_lo2"] = plan.idx_lo[1, c]
        m["idx_hi2"] = plan.idx_hi[1, c]
        m["oh"] = plan.oh[c]
        m["ohT"] = plan.ohT[c]
        if plan.subcnt is not None:
            m["subcnt"] = plan.subcnt[c:c + 1]
        in_maps.append(m)
    return in_maps


# ----------------------------------------------------------------------------
# Harness entry point
# ----------------------------------------------------------------------------

import os


def _ensure_ntff_hook():
    import sys
    import types
    try:
        import antenv.axon_hooks  # noqa: F401
        return
    except ImportError:
        pass
    mod = types.ModuleType("antenv.axon_hooks")
    state = {}
    mod.set_axon_ntff_profile_hook = lambda h: state.__setitem__("h", h)
    mod.get_axon_ntff_profile_hook = lambda: state.get("h")
    import antenv
    sys.modules["antenv.axon_hooks"] = mod
    antenv.axon_hooks = mod
    try:
        from trn_agent_boot.trn_boot import _ntff_profile_via_ctypes
        hook = _ntff_profile_via_ctypes("/opt/axon/libaxon_pjrt.so")
        if hook is not None:
            mod.set_axon_ntff_profile_hook(hook)
    except Exception as e:  # noqa: BLE001
        print("ntff hook setup failed:", e)


def _build_nc(plan):
    import concourse.bacc as bacc
    nc = bacc.Bacc("TRN2", target_bir_lowering=False, debug=False,
                   num_devices=plan.n_cores, num_swdge_queues=4)
    ins_t = {
        "xT": nc.dram_tensor("xT", [128, plan.N], BF16, kind="ExternalInput").ap(),
        "W1aug": nc.dram_tensor("W1aug", [128, 192], BF16, kind="ExternalInput").ap(),
        "W2m": nc.dram_tensor("W2m", [128, 66], BF16, kind="ExternalInput").ap(),
        "ident": nc.dram_tensor("ident", [128, 128], BF16, kind="ExternalInput").ap(),
        "idx_lo1": nc.dram_tensor("idx_lo1", [128, plan.TOT_LO // 16], I16,
                                  kind="ExternalInput").ap(),
        "idx_hi1": nc.dram_tensor("idx_hi1", [128, plan.TOT_HI // 16], I16,
                                  kind="ExternalInput").ap(),
        "idx_lo2": nc.dram_tensor("idx_lo2", [128, plan.TOT_LO // 16], I16,
                                  kind="ExternalInput").ap(),
        "idx_hi2": nc.dram_tensor("idx_hi2", [128, plan.TOT_HI // 16], I16,
                                  kind="ExternalInput").ap(),
        "xT_own": nc.dram_tensor("xT_own", [128, plan.Nc], BF16,
                                 kind="ExternalInput").ap(),
        "oh": nc.dram_tensor("oh", [128, plan.S * 128], FP8,
                             kind="ExternalInput").ap(),
        "ohT": nc.dram_tensor("ohT", [128, plan.S * 128], FP8,
                              kind="ExternalInput").ap(),
    }
    if plan.subcnt is not None:
        ins_t["subcnt"] = nc.dram_tensor(
            "subcnt", [1, plan.subcnt.shape[1]], mybir.dt.int32,
            kind="ExternalInput").ap()
    outs_t = {
        "out2": nc.dram_tensor("out2", [plan.Nc, 64], FP32,
                               kind="ExternalOutput").ap(),
    }
    with tile.TileContext(nc) as t:
        emit_gat(t, outs_t, ins_t, plan)
    nc.compile()
    return nc


def kernel(**inputs):
    global LAST_RESULT
    from concourse.bass_utils import run_bass_kernel_spmd

    x = np.asarray(inputs["x"], np.float32)
    edge_index = np.asarray(inputs["edge_index"])
    W1 = np.asarray(inputs["W1"], np.float32)
    as1 = np.asarray(inputs["att_src1"], np.float32)
    ad1 = np.asarray(inputs["att_dst1"], np.float32)
    b1 = np.asarray(inputs["b1"], np.float32)
    W2 = np.asarray(inputs["W2"], np.float32)
    as2 = np.asarray(inputs["att_src2"], np.float32)
    ad2 = np.asarray(inputs["att_dst2"], np.float32)
    b2 = np.asarray(inputs["b2"], np.float32)
    assert float(np.abs(b1).max()) == 0.0, "nonzero b1 not supported"

    N = x.shape[0]
    plan = make_plan(edge_index, N, N_CORES, group_windows=1)
    in_maps = build_host_inputs(plan, x, W1, as1, ad1, W2, as2, ad2)
    nc = _build_nc(plan)
    trace = os.environ.get("GAT_TRACE", "0") == "1"
    if trace:
        _ensure_ntff_hook()
    res = run_bass_kernel_spmd(nc, in_maps, core_ids=list(range(plan.n_cores)),
                               trace=trace)
    LAST_RESULT = res
    out = np.concatenate([res.results[c]["out2"] for c in range(plan.n_cores)],
                         axis=0)
    return (out + b2[None, :]).astype(np.float32)
